# revision 1
# baseline (speedup 1.0000x reference)
"""CLAHE (nn_EqualizeClahe) Trainium2 Bass kernel.

kernel(x): x (8,3,1024,1024) fp32 in [0,1) -> same-shape output.
8 NeuronCores data parallel: core i processes image i (3 channels).

Per channel (1024x1024, 8x8 grid of 128x128 tiles):
  prep:  b = trunc(x*256) (exact: x*256 is an exact fp32 product for
         x = k*2^-24), split b = b16 + lo nibbles; idx = trunc(x*255)
         (matches reference's int32 truncation of the identical product).
  hist:  per tile, 256 bins as a 16x16 outer product accumulated on the
         TensorEngine: for each of the 128 pixel-columns c of a tile,
         psum[16,16] += OH_c^T @ OL_c, with OH/OL 16-wide one-hots of
         hi/lo built by DVE is_equal slabs (bf16).
  lut:   clip at 2560; the uniform redistribution is applied analytically
         after the cumsum: C~[i] = C[i] + (i+1)*base + min(i+1, residual).
         Cumsum = shift-add doubling within the 16 lo-bins + triangular
         matmul across the 16 hi-rows.  All integer arithmetic is exact
         in fp32 (sums <= 16384; 255/16384 is a power-of-two scaled int).
  apply: per pixel, the 4 neighbour-tile LUT values come from one
         GPSIMD ap_gather on a quad-interleaved bf16 table (d=4), then
         a bilinear blend on DVE and division by 255.

Self-contained: only needs /opt/trn_rl_repo (concourse) + numpy.
"""
import sys

for _p in ("/opt/trn_rl_repo",):
    if _p not in sys.path:
        sys.path.insert(0, _p)

import dataclasses
from contextlib import ExitStack

import numpy as np

import concourse.bass as bass
import concourse.mybir as mybir
import concourse.tile as tile
from concourse.bass_utils import run_bass_kernel_spmd

FP32 = mybir.dt.float32
BF16 = mybir.dt.bfloat16
I16 = mybir.dt.int16
OP = mybir.AluOpType

H = W = 1024
CH = 3
NB = 256
TS = 128
PIX = TS * TS
MAXV = 2560.0
SCALE = float((NB - 1) / PIX)

_CACHE = {}


# ----------------------------------------------------------------- helpers
def _bcast(ap, dim_counts):
    """Append step-0 dims (broadcast) to an AP: dim_counts = [(pos_ignored, n)...]"""
    new = list(ap.ap) + [[0, n] for n in dim_counts]
    return dataclasses.replace(ap, ap=new)


def _interp_coords(n_tiles, tile_size, length):
    half = tile_size // 2
    pos = np.arange(length)
    j = pos // half
    p = pos % half
    r0 = np.clip((j - 1) // 2, 0, n_tiles - 1)
    r1 = np.clip(r0 + 1, 0, n_tiles - 1)
    denom = np.float32(2 * half - 1)
    w = np.where(j % 2 == 1, (2 * half - 1) - p, (half - 1) - p).astype(np.float32) / denom
    w = np.where(j == 0, np.float32(1.0), w).astype(np.float32)
    return r0, r1, w


def _quad_moves():
    """(band -> list of (tile_idx, cell_block)) for quad table construction.

    Table entry layout per band: element e = (jyr*16+jx)*256 + i holds 4
    interleaved bf16 values g=0..3 = LUT[tile(g)][i]."""
    r0, r1, _ = _interp_coords(8, TS, H)
    c0, c1, _ = _interp_coords(8, TS, W)
    ry0, ry1 = r0[::64], r1[::64]      # per cell row (16)
    cx0, cx1 = c0[::64], c1[::64]
    plans = []
    for a in range(8):
        moves = []
        for jyr in range(2):
            jy = 2 * a + jyr
            for jx in range(16):
                quad = ((ry0[jy], cx0[jx]), (ry0[jy], cx1[jx]),
                        (ry1[jy], cx0[jx]), (ry1[jy], cx1[jx]))
                for g, (ty, tx) in enumerate(quad):
                    moves.append((int(ty) * 8 + int(tx), (jyr * 16 + jx), g))
        plans.append(moves)
    return plans


def _host_consts():
    c = {}
    c["ltri"] = np.tril(np.ones((16, 16), np.float32), -1)        # lhsT[K=16,M=16]: L[k,m]=1 iff k<m... see below
    # we need pre[j] = sum_{j'<j} rowtot[j'] = sum_k lhsT[k,j]*rt[k] with lhsT[k,j]=1 iff k<j
    c["ltri"] = np.triu(np.ones((16, 16), np.float32), 1)          # [K=16(k), M=16(j)] 1 iff k<j
    c["ones16"] = np.ones((16, 1), np.float32)
    c["iota1"] = (np.arange(256, dtype=np.float32).reshape(16, 16) + 1.0)
    rowhalf = (np.arange(128) // 64).astype(np.int32)
    jx = (np.arange(W) // 64).astype(np.int32)
    c["gbase"] = ((rowhalf[:, None] * 16 + jx[None, :]) * 256 * 4).astype(np.float32)
    r0, r1, wy = _interp_coords(8, TS, H)
    c0_, c1_, wx = _interp_coords(8, TS, W)
    c["wy"] = np.ascontiguousarray(wy.reshape(8, 128).T)           # [128, 8]
    c["wx"] = np.ascontiguousarray(np.broadcast_to(wx[None, :], (128, W)))
    return c


# ----------------------------------------------------------------- kernel IR
def _emit(nc, tc, ctx, x_in, y_out, K):
    quad_plans = _quad_moves()
    pool = ctx.enter_context(tc.tile_pool(name="main", bufs=1))
    gpool = ctx.enter_context(tc.tile_pool(name="gq", bufs=4))
    pspool = ctx.enter_context(tc.tile_pool(name="ps", bufs=2, space="PSUM"))
    ps1pool = ctx.enter_context(tc.tile_pool(name="ps1", bufs=1, space="PSUM"))

    # constants
    ltri = pool.tile([16, 16], FP32, tag="ltri")
    nc.sync.dma_start(ltri[:], K["ltri"].ap())
    ones16 = pool.tile([16, 1], FP32, tag="ones16")
    nc.sync.dma_start(ones16[:], K["ones16"].ap())
    iota1 = pool.tile([16, 16], FP32, tag="iota1")
    nc.sync.dma_start(iota1[:], K["iota1"].ap())
    gb = pool.tile([128, W], FP32, tag="gbase")
    nc.sync.dma_start(gb[:], K["gbase"].ap())
    wyt = pool.tile([128, 8], FP32, tag="wy")
    nc.sync.dma_start(wyt[:], K["wy"].ap())
    wxt = pool.tile([128, W], FP32, tag="wx")
    nc.sync.dma_start(wxt[:], K["wx"].ap())

    def prep_band(ch, a, want_flat):
        """load band a, return (lo, b16) or flat tiles (bf16/int16)."""
        xband = pool.tile([128, W], FP32, tag="xband")
        nc.sync.dma_start(xband[:], x_in[ch, a * 128:(a + 1) * 128, :])
        scrA = pool.tile([128, W], FP32, tag="scrA")
        scrB = pool.tile([128, W], FP32, tag="scrB")
        if not want_flat:
            # b = trunc(x*256); lo = mod(b,16); b16 = b - lo
            scrI = pool.tile([128, W], mybir.dt.int32, tag="scrI")
            nc.vector.tensor_scalar(scrA[:], xband[:], 256.0, None, op0=OP.mult)
            nc.vector.tensor_copy(scrI[:], scrA[:])
            nc.vector.tensor_copy(scrB[:], scrI[:])
            fx = pool.tile([128, W], FP32, tag="fx")
            nc.vector.tensor_tensor(fx[:], scrB[:], scrA[:], op=OP.is_gt)
            nc.vector.tensor_tensor(scrB[:], scrB[:], fx[:], op=OP.subtract)  # b
            # hi*16 via floor(b/16)
            nc.vector.tensor_scalar(scrA[:], scrB[:], 0.0625, None, op0=OP.mult)
            nc.vector.tensor_copy(scrI[:], scrA[:])
            fx2 = pool.tile([128, W], FP32, tag="fx2")
            nc.vector.tensor_copy(fx2[:], scrI[:])
            nc.vector.tensor_tensor(fx[:], fx2[:], scrA[:], op=OP.is_gt)
            nc.vector.tensor_tensor(fx2[:], fx2[:], fx[:], op=OP.subtract)    # hi
            b16 = pool.tile([128, W], BF16, tag="b16")
            nc.vector.tensor_scalar(b16[:], fx2[:], 16.0, None, op0=OP.mult)
            lo = pool.tile([128, W], BF16, tag="lo")
            nc.vector.tensor_tensor(lo[:], scrB[:], b16[:], op=OP.subtract)
            return lo, b16
        # idx = trunc(x*255); flat = idx + gbase
        scrI = pool.tile([128, W], mybir.dt.int32, tag="scrI")
        nc.vector.tensor_scalar(scrA[:], xband[:], 255.0, None, op0=OP.mult)
        nc.vector.tensor_copy(scrI[:], scrA[:])
        nc.vector.tensor_copy(scrB[:], scrI[:])
        fx = pool.tile([128, W], FP32, tag="fx")
        nc.vector.tensor_tensor(fx[:], scrB[:], scrA[:], op=OP.is_gt)
        nc.vector.tensor_tensor(scrB[:], scrB[:], fx[:], op=OP.subtract)  # idx fp32
        flat = pool.tile([128, W], mybir.dt.uint16, tag="flat")
        nc.vector.scalar_tensor_tensor(flat[:], scrB[:], 4.0, gb[:],
                                       op0=OP.mult, op1=OP.add)
        return flat

    for ch in range(CH):
        # ---------------- histogram ----------------
        hsb = pool.tile([16, 64 * 16], FP32, tag="hsb")
        for a in range(8):
            lo, b16 = prep_band(ch, a, want_flat=False)
            for q in range(4):          # quarter-bands of 256 cols = 2 tiles
                ohh = pool.tile([128, 16 * 256], BF16, tag="ohh")
                ohl = pool.tile([128, 16 * 256], BF16, tag="ohl")
                for j in range(16):
                    nc.vector.tensor_scalar(ohh[:, j * 256:(j + 1) * 256],
                                            b16[:, q * 256:(q + 1) * 256],
                                            float(16 * j), None, op0=OP.is_equal)
                    nc.vector.tensor_scalar(ohl[:, j * 256:(j + 1) * 256],
                                            lo[:, q * 256:(q + 1) * 256],
                                            float(j), None, op0=OP.is_equal)
                oh3 = ohh[:].rearrange("p (j x) -> p j x", j=16)
                ol3 = ohl[:].rearrange("p (j x) -> p j x", j=16)
                for sub in range(2):    # 2 tiles per quarter
                    ps = pspool.tile([16, 16], FP32, tag="hps")
                    for cc in range(128):
                        col = sub * 128 + cc
                        nc.tensor.matmul(ps[:], oh3[:, :, col], ol3[:, :, col],
                                         start=(cc == 0), stop=(cc == 127))
                    t = a * 8 + q * 2 + sub
                    nc.vector.tensor_scalar(hsb[:, t * 16:(t + 1) * 16], ps[:],
                                            MAXV, None, op0=OP.min)

        # ---------------- LUT build [16, (t,k)] ----------------
        r1t = pool.tile([16, 64 * 16], FP32, tag="r1")
        r2t = pool.tile([16, 64 * 16], FP32, tag="r2")

        def shift_add(dst, src, s):
            nc.vector.tensor_copy(dst[:], src[:])
            d3 = dst[:].rearrange("p (t k) -> p t k", k=16)[:, :, s:]
            s3 = src[:].rearrange("p (t k) -> p t k", k=16)[:, :, :16 - s]
            nc.vector.tensor_tensor(d3, d3, s3, op=OP.add)

        shift_add(r1t, hsb, 1)
        shift_add(r2t, r1t, 2)
        shift_add(r1t, r2t, 4)
        shift_add(r2t, r1t, 8)

        rt = r2t[:].rearrange("p (t k) -> p t k", k=16)[:, :, 15]
        pre_ps = ps1pool.tile([16, 64], FP32, tag="pre")
        nc.tensor.matmul(pre_ps[:], ltri[:], rt, start=True, stop=True)
        tot_ps = ps1pool.tile([1, 64], FP32, tag="tot")
        nc.tensor.matmul(tot_ps[:], ones16[:], rt, start=True, stop=True)
        tot = pool.tile([1, 64], FP32, tag="tot")
        nc.vector.tensor_copy(tot[:], tot_ps[:])
        o1 = pool.tile([1, 16], FP32, tag="o1")
        nc.vector.memset(o1[:], 1.0)
        tot16_ps = ps1pool.tile([16, 64], FP32, tag="tot16")
        nc.tensor.matmul(tot16_ps[:], o1[:], tot[:], start=True, stop=True)

        clip16 = pool.tile([16, 64], FP32, tag="clip16")
        nc.vector.tensor_scalar(clip16[:], tot16_ps[:], -1.0, 16384.0,
                                op0=OP.mult, op1=OP.add)
        basev = pool.tile([16, 64], FP32, tag="basev")
        nc.vector.tensor_scalar(basev[:], clip16[:], 1.0 / 256.0, None, op0=OP.mult)
        ri = pool.tile([16, 64], mybir.dt.int32, tag="ri")
        nc.vector.tensor_copy(ri[:], basev[:])
        rf = pool.tile([16, 64], FP32, tag="rf")
        nc.vector.tensor_copy(rf[:], ri[:])
        resid = pool.tile([16, 64], FP32, tag="resid")
        nc.vector.tensor_tensor(resid[:], rf[:], basev[:], op=OP.is_gt)
        nc.vector.tensor_tensor(basev[:], rf[:], resid[:], op=OP.subtract)  # base=floor
        nc.vector.scalar_tensor_tensor(resid[:], basev[:], -256.0, clip16[:],
                                       op0=OP.mult, op1=OP.add)             # resid

        ct = r2t[:].rearrange("p (t k) -> p t k", k=16)
        pre = pool.tile([16, 64], FP32, tag="presb")
        nc.vector.tensor_copy(pre[:], pre_ps[:])
        nc.vector.tensor_tensor(ct, ct, _bcast(pre[:], [16]), op=OP.add)
        tmp = pool.tile([16, 64 * 16], FP32, tag="tmpc")
        tmp3 = tmp[:].rearrange("p (t k) -> p t k", k=16)
        iota_b = dataclasses.replace(iota1[:], ap=[iota1[:].ap[0], [0, 64], iota1[:].ap[1]])
        nc.vector.tensor_tensor(tmp3, iota_b, _bcast(basev[:], [16]), op=OP.mult)
        nc.vector.tensor_tensor(ct, ct, tmp3, op=OP.add)
        nc.vector.tensor_tensor(tmp3, iota_b, _bcast(resid[:], [16]), op=OP.min)
        nc.vector.tensor_tensor(ct, ct, tmp3, op=OP.add)

        nc.vector.tensor_scalar(r2t[:], r2t[:], SCALE, None, op0=OP.mult)
        li = pool.tile([16, 64 * 16], mybir.dt.int32, tag="li")
        nc.vector.tensor_copy(li[:], r2t[:])
        nc.vector.tensor_copy(r1t[:], li[:])
        lfx = pool.tile([16, 64 * 16], FP32, tag="lfx")
        nc.vector.tensor_tensor(lfx[:], r1t[:], r2t[:], op=OP.is_gt)
        lutb = pool.tile([16, 64 * 16], BF16, tag="lutb")
        nc.vector.tensor_tensor(lutb[:], r1t[:], lfx[:], op=OP.subtract)

        # ---------------- apply ----------------
        for a in range(8):
            flat = prep_band(ch, a, want_flat=True)
            trep = pool.tile([128, 32 * 256 * 4], BF16, tag="trep")
            d4 = trep[0:1, :].rearrange("o (e g) -> o e g", g=4)
            for (tile_idx, cell, g) in quad_plans[a]:
                # src: lutb [16, 16] block of tile tile_idx (j on partitions)
                # dst: 256 entries strided by 4 on trep partition 0
                nc.sync.dma_start(d4[:, cell * 256:(cell + 1) * 256, g],
                                  lutb[:, tile_idx * 16:(tile_idx + 1) * 16])
            for k in range(7):   # replicate partition 0 to all 128 (log doubling)
                n = 1 << k
                nc.sync.dma_start(trep[n:2 * n, :], trep[0:n, :])

            gscr = K["gscr"].ap()
            for q in range(64):
                gout = gpool.tile([128, 256 * 4], BF16, tag="gout")
                nc.gpsimd.indirect_copy(
                    gout[:].rearrange("p (i d) -> p i d", d=4),
                    trep[:].rearrange("p (e d) -> p e d", d=4),
                    flat[:, q * 16:(q + 1) * 16], True)
                # keep one partition per core (values replicated within core)
                nc.sync.dma_start(gscr[:, q * 1024:(q + 1) * 1024], gout[0::16, :])
            gpx = pool.tile([128, W * 4], BF16, tag="gpx")
            for j in range(16):
                # pixel (16k+j, x=q*16+s) at gscr[k, q*1024 + (s*16+j)*4 + g]
                srcj = gscr[:, j * 4:].rearrange("p (q s g) -> p q s g",
                                                 q=64, s=16) if False else None
                import dataclasses as _dc
                sap = gscr[:, :]
                sap = _dc.replace(sap, offset=sap.offset + j * 4,
                                  ap=[sap.ap[0], [1024, 64], [64, 16], [1, 4]])
                dstj = gpx[j::16, :].rearrange("p (q s g) -> p q s g", q=64, s=16)
                nc.sync.dma_start(dstj, sap)

            g4 = gpx[:].rearrange("p (x g) -> p x g", g=4)
            g00, g01, g10, g11 = g4[:, :, 0], g4[:, :, 1], g4[:, :, 2], g4[:, :, 3]
            d0 = pool.tile([128, W], FP32, tag="bd0")
            nc.vector.tensor_tensor(d0[:], g00, g01, op=OP.subtract)
            nc.vector.tensor_tensor(d0[:], d0[:], wxt[:], op=OP.mult)
            nc.vector.tensor_tensor(d0[:], d0[:], g01, op=OP.add)
            d1 = pool.tile([128, W], FP32, tag="bd1")
            nc.vector.tensor_tensor(d1[:], g10, g11, op=OP.subtract)
            nc.vector.tensor_tensor(d1[:], d1[:], wxt[:], op=OP.mult)
            nc.vector.tensor_tensor(d1[:], d1[:], g11, op=OP.add)
            nc.vector.tensor_tensor(d0[:], d0[:], d1[:], op=OP.subtract)
            res = pool.tile([128, W], FP32, tag="scrA")
            nc.vector.scalar_tensor_tensor(res[:], d0[:], wyt[:, a:a + 1], d1[:],
                                           op0=OP.mult, op1=OP.add)
            nc.vector.tensor_scalar(res[:], res[:], float(np.float32(1.0) / np.float32(255.0)), None, op0=OP.mult)
            nc.sync.dma_start(y_out[ch, a * 128:(a + 1) * 128, :], res[:])


def _bcast_part(ap, n):
    """Replicate a [1, F] AP across n partitions (partition step 0)."""
    new = [[0, n]] + list(ap.ap[1:])
    return dataclasses.replace(ap, ap=new)


def _apply_tile_patch():
    """This walrus build rejects >2 sync waits on one instruction; split the
    TileContext exit drain's waits into individual nops."""
    def _patched(self, tick_clock, wait_clock):
        nc = self.nc
        probe = nc.sync.nop()
        wait_clock.add_sem_waits(probe.ins,
                                 tile.ScopedClock({None: tick_clock.global_clock}))
        si = probe.ins.sync_info
        waits = list(si.on_wait) if si and si.on_wait else []
        if len(waits) > 1:
            probe.ins.sync_info = mybir.SyncInfo(on_wait=[waits[0]], on_update=[])
            for w in waits[1:]:
                extra = nc.sync.nop()
                extra.ins.sync_info = mybir.SyncInfo(on_wait=[w], on_update=[])
        nc.sync.drain()
        nc.all_engine_barrier()
        assert self.sems is not None
        popped = nc._tile_sem_poison_stack.pop()
        assert popped is self._sem_poison
        nc.clear_and_free_semaphores(list(self.sems.allocated().values()))
        nc.all_engine_barrier()
    tile.TileContext._drain_and_barrier = _patched




def _split_waits(nc, maxw=1):
    """This container's walrus rejects instructions with more than ~2 sem
    waits; hoist excess waits onto same-engine NoOps inserted just before."""
    import bass_rust
    counter = [0]
    for f in nc.m.functions:
        for blk in f.blocks:
            insts = blk.instructions
            out = []
            for ins in insts:
                si = ins.sync_info
                waits = list(si.on_wait) if si and si.on_wait else []
                if len(waits) > maxw:
                    keep = waits[:maxw]
                    extra = waits[maxw:]
                    for w in extra:
                        counter[0] += 1
                        nop = bass_rust.InstNoOp(
                            name=f"WSPLIT-{counter[0]}", engine=ins.engine,
                            ins=[], outs=[],
                            sync_info=mybir.SyncInfo(on_wait=[w], on_update=[]))
                        out.append(nop)
                    ins.sync_info = mybir.SyncInfo(
                        on_wait=keep, on_update=list(si.on_update or []))
                out.append(ins)
            blk.instructions = out

def build():
    if "nc" in _CACHE:
        return _CACHE["nc"]
    _apply_tile_patch()
    nc = bass.Bass("TRN2", target_bir_lowering=False, debug=False)
    x_in = nc.dram_tensor("x", [CH, H, W], FP32, kind="ExternalInput").ap()
    y_out = nc.dram_tensor("y", [CH, H, W], FP32, kind="ExternalOutput").ap()
    hk = _host_consts()
    K = {k: nc.inline_tensor(v, name=f"const_{k}") for k, v in hk.items()}
    K["gscr"] = nc.dram_tensor("gscr", [8, 64 * 1024], BF16)
    with ExitStack() as ctx:
        tc = ctx.enter_context(tile.TileContext(nc))
        _emit(nc, tc, ctx, x_in, y_out, K)
    _split_waits(nc)
    _CACHE["nc"] = nc
    return nc


def kernel(x: np.ndarray) -> np.ndarray:
    x = np.ascontiguousarray(np.asarray(x, dtype=np.float32))
    assert x.shape == (8, CH, H, W), x.shape
    nc = build()
    in_maps = [{"x": x[i]} for i in range(8)]
    res = run_bass_kernel_spmd(nc, in_maps, list(range(8)))
    out = np.stack([res.results[i]["y"] for i in range(8)], axis=0)
    return out.astype(np.float32)


if __name__ == "__main__":
    xs = np.load("/tmp/x_full.npy") if False else None
    x = np.random.rand(8, CH, H, W).astype(np.float32)
    y = kernel(x)
    print("ran:", y.shape, y.dtype)



# revision 30
# speedup vs baseline: 73.4199x; 73.4199x over previous
"""CLAHE (nn_EqualizeClahe) Trainium2 Bass kernel, v2.

kernel(x): x (8,3,1024,1024) fp32 in [0,1) -> same-shape output.
8 NeuronCores data parallel: core i processes image i (3 channels).

Per channel (1024x1024, 8x8 grid of 128x128 tiles):
  prep:  u    = round(x*256 + 0.5)  (= bin+1, exact for x = k*2^-24)
         uidx = round(x*255 + 0.5)  (= lut index + 1)
  hist:  per tile, 256 bins as a 16x16 (hi x lo) outer product accumulated
         on the TensorEngine, one column-matmul per pixel column; hi/lo
         one-hots built by DVE is_equal slabs in bf16.
  lut:   hist flattened to [tile(64 partitions), 256 bins]; clip at 2560,
         cumsum by shift-add doubling along the free dim, uniform
         redistribution of the clipped excess, floored scale to final
         LUT/255 in bf16.
  apply: ONE indirect_copy per half-band: each GPSIMD core's 16 partitions
         hold the 16 neighbour-tile LUTs (2 tile rows x 8 tile cols) its
         16 pixel rows may reference; the raw uidx tensor is the index
         stream, so every pixel is looked up through all 16 LUTs at once.
         A static wx-weight multiply (DVE), then 16 small matmuls whose
         lhsT constants carry the wy weights reduce each core's 16
         partitions into the final blended pixel, stacked 3 k-groups per
         PSUM tile (bases 0/32/64).  Two [128,512] copies per psum tile
         evacuate to SBUF and one strided DMA per (band, k-mod-3 group)
         writes DRAM in row order.

Self-contained: only needs /opt/trn_rl_repo (concourse) + numpy.
"""
import sys

for _p in ("/opt/trn_rl_repo",):
    if _p not in sys.path:
        sys.path.insert(0, _p)

import dataclasses
from contextlib import ExitStack

import numpy as np

import concourse.bass as bass
import concourse.mybir as mybir
import concourse.tile as tile
from concourse.bass_utils import run_bass_kernel_spmd

FP32 = mybir.dt.float32
BF16 = mybir.dt.bfloat16
U16 = mybir.dt.uint16
U8 = mybir.dt.uint8
OP = mybir.AluOpType
ACT = mybir.ActivationFunctionType

H = W = 1024
CH = 3
NB = 256
TS = 128          # tile size (8x8 grid)
MAXV = 2560.0     # clip limit * pixels / bins
LUT_SCALE = float(np.float32(255.0 / 16384.0))

_CACHE = {}


def _bcast_free(ap, n):
    """[P, 1] -> [P, n] via a step-0 free dim."""
    new = [ap.ap[0], [0, n]]
    return dataclasses.replace(ap, ap=new)


def _interp_coords(n_tiles, tile_size, length):
    half = tile_size // 2
    pos = np.arange(length)
    j = pos // half
    p = pos % half
    r0 = np.clip((j - 1) // 2, 0, n_tiles - 1)
    r1 = np.clip(r0 + 1, 0, n_tiles - 1)
    denom = np.float32(2 * half - 1)
    w = np.where(j % 2 == 1, (2 * half - 1) - p, (half - 1) - p).astype(np.float32) / denom
    w = np.where(j == 0, np.float32(1.0), w).astype(np.float32)
    return r0, r1, w


def _host_consts():
    import ml_dtypes
    c0, c1, wx = _interp_coords(8, TS, W)
    r0, r1, wy = _interp_coords(8, TS, H)

    # WX[p, 16*x + k] = wx-role weight of tile-col (p%8) at column x
    wxrow = np.zeros((8, W), np.float32)
    for cp in range(8):
        wxrow[cp] = wx * (c0 == cp) + (1.0 - wx) * (c1 == cp)
    wx_full = np.zeros((128, W, 16), np.float32)
    for p in range(128):
        wx_full[p, :, :] = wxrow[p % 8][:, None]
    wx_full = wx_full.reshape(128, 16 * W)

    # L[p, (a*16+k)*8 + c]: wy-weighted core-reduction lhsT
    lab = np.zeros((128, 8 * 16 * 8), np.float32)
    for a in range(8):
        for k in range(16):
            for c in range(8):
                y = 128 * a + 16 * c + k
                for p in range(16 * c, 16 * c + 16):
                    s = (p % 16) // 8
                    lab[p, (a * 16 + k) * 8 + c] = wy[y] if s == 0 else 1.0 - wy[y]

    iota1 = np.broadcast_to(np.arange(1, NB + 1, dtype=np.float32)[None, :],
                            (64, NB)).copy()
    return {
        "wx": wx_full.astype(ml_dtypes.bfloat16),
        "lab": lab.astype(ml_dtypes.bfloat16),
        "iota1": iota1,
    }


# ----------------------------------------------------------------- kernel IR
def _emit(nc, tc, ctx, x_in, y_out, K):
    pool = ctx.enter_context(tc.tile_pool(name="main", bufs=1))
    pool2 = ctx.enter_context(tc.tile_pool(name="dbuf", bufs=2))
    pool4 = ctx.enter_context(tc.tile_pool(name="qbuf", bufs=3))
    pshist = ctx.enter_context(tc.tile_pool(name="pshist", bufs=2, space="PSUM"))
    psred = ctx.enter_context(tc.tile_pool(name="psred", bufs=1, space="PSUM"))

    wxb = pool.tile([128, 16 * W], BF16, tag="wxb")
    nc.sync.dma_start(wxb[:], K["wx"].ap())
    lab = pool.tile([128, 1024], BF16, tag="lab")
    nc.sync.dma_start(lab[:], K["lab"].ap())
    iot = pool.tile([64, NB], FP32, tag="iot")
    nc.sync.dma_start(iot[:], K["iota1"].ap())

    def hist_prep(ch, a, uidx):
        """Load band, produce hbf (hi one-hot input), lo (lo+1), store uidx u8."""
        xb = pool2.tile([128, W], FP32, tag="xb", name=f"xb_{ch}_{a}")
        nc.sync.dma_start(xb[:], x_in[ch, a * 128:(a + 1) * 128, :])
        u1 = pool2.tile([128, W], U16, tag="u1", name=f"u1_{ch}_{a}")
        nc.scalar.activation(u1[:], xb[:], ACT.Copy, bias=0.5, scale=256.0)
        nc.scalar.activation(uidx[:, a * W:(a + 1) * W], xb[:], ACT.Copy,
                             bias=0.5 + 2.0 ** -12, scale=255.0)
        ubf = pool2.tile([128, W], BF16, tag="ubf", name=f"ubf_{ch}_{a}")
        nc.vector.tensor_copy(ubf[:], u1[:])
        hiu = pool2.tile([128, W], U16, tag="hiu", name=f"hiu_{ch}_{a}")
        nc.scalar.activation(hiu[:], ubf[:], ACT.Copy, bias=-0.5525, scale=0.0625)
        hbf = pool2.tile([128, W], BF16, tag="hbf", name=f"hbf_{ch}_{a}")
        nc.vector.tensor_copy(hbf[:], hiu[:])
        lo = pool2.tile([128, W], BF16, tag="lo", name=f"lo_{ch}_{a}")
        nc.vector.scalar_tensor_tensor(lo[:], hbf[:], -16.0, ubf[:],
                                       op0=OP.mult, op1=OP.add)  # lo+1 in 1..16
        return hbf, lo

    def hist_quarter(ch, a, qd, hbf, lo, hp):
        # half-band slabs (qd = 0, 2 start a half; qd = 1, 3 run its 2nd tile pair)
        if qd % 2 == 0:
            h = qd // 2
            ohh = pool.tile([128, 16 * 512], BF16, tag="ohh")
            ohl = pool.tile([128, 16 * 512], BF16, tag="ohl")
            for j in range(16):
                nc.vector.tensor_scalar(ohh[:, j * 512:(j + 1) * 512],
                                        hbf[:, h * 512:(h + 1) * 512],
                                        float(j), None, op0=OP.is_equal)
                nc.vector.tensor_scalar(ohl[:, j * 512:(j + 1) * 512],
                                        lo[:, h * 512:(h + 1) * 512],
                                        float(j + 1), None, op0=OP.is_equal)
            hist_quarter.slabs = (ohh, ohl)
        ohh, ohl = hist_quarter.slabs
        oh3 = ohh[:].rearrange("p (j x) -> p j x", j=16)
        ol3 = ohl[:].rearrange("p (j x) -> p j x", j=16)
        for t in range(2):
            T = 2 * qd + t
            xt = (2 * qd + t) % 4
            for col in range(128):
                xl = 128 * xt + col
                nc.tensor.matmul(hp[:, 16 * T:16 * T + 16],
                                 oh3[:, :, xl], ol3[:, :, xl],
                                 start=(col == 0), stop=(col == 127))

    def hsb_flush(ch, a, hp, hsb, ht):
        nc.scalar.copy(hsb[:, a * 128:(a + 1) * 128], hp[:])
        for tt_ in range(8):
            t = 8 * a + tt_
            nc.sync.dma_start(ht[t:t + 1, :],
                              hsb[0:16, a * 128 + 16 * tt_: a * 128 + 16 * tt_ + 16])

    def lut_build(ch, hsb, ht):
        ca = pool.tile([64, NB], FP32, tag="ca")
        cb = pool.tile([64, NB], FP32, tag="cb")
        nc.vector.tensor_scalar(ca[:], ht[:], MAXV, None, op0=OP.min)
        cur, nxt = ca, cb
        for s in (1, 2, 4, 8, 16, 32, 64, 128):
            nc.vector.tensor_copy(nxt[:, :s], cur[:, :s])
            nc.vector.tensor_tensor(nxt[:, s:], cur[:, s:], cur[:, :NB - s], op=OP.add)
            cur, nxt = nxt, cur
        # cur = cumsum C; redistribution
        clip16 = pool.tile([64, 1], FP32, tag="clip16")
        nc.vector.tensor_scalar(clip16[:], cur[:, NB - 1:NB], -1.0, 16384.0,
                                op0=OP.mult, op1=OP.add)
        bsf = pool.tile([64, 1], FP32, tag="bsf")
        nc.vector.tensor_scalar(bsf[:], clip16[:], 1.0 / 256.0, -0.49,
                                op0=OP.mult, op1=OP.add)
        bsu = pool.tile([64, 1], U16, tag="bsu")
        nc.scalar.copy(bsu[:], bsf[:])
        nc.scalar.copy(bsf[:], bsu[:])  # base, exact fp32
        resid = pool.tile([64, 1], FP32, tag="resid")
        nc.vector.scalar_tensor_tensor(resid[:], bsf[:], -256.0, clip16[:],
                                       op0=OP.mult, op1=OP.add)
        tmp = pool.tile([64, NB], FP32, tag="tmp")
        nc.vector.tensor_tensor(tmp[:], iot[:], _bcast_free(bsf[:], NB), op=OP.mult)
        nc.vector.tensor_tensor(nxt[:], cur[:], tmp[:], op=OP.add)
        nc.vector.tensor_tensor(tmp[:], iot[:], _bcast_free(resid[:], NB), op=OP.min)
        nc.vector.tensor_tensor(cur[:], nxt[:], tmp[:], op=OP.add)
        # LUT = floor(C~ * 255/16384) / 255, fp32
        nc.vector.tensor_scalar(nxt[:], cur[:], LUT_SCALE, -0.499,
                                op0=OP.mult, op1=OP.add)
        lu16 = pool.tile([64, NB], U16, tag="lu16")
        nc.scalar.copy(lu16[:], nxt[:])
        ltb0 = pool.tile([64, NB], FP32, tag="ltb0")
        nc.scalar.copy(ltb0[:], lu16[:])
        lt = pool.tile([64, NB], FP32, tag="lt")
        nc.vector.tensor_scalar(lt[:], ltb0[:], float(np.float32(1.0) / np.float32(255.0)),
                                None, op0=OP.mult)
        # TB [128, 8*257]: per band a, partition 16c+8s+c' holds the LUT of
        # tile (rowpair(a, c//4, s), c'), entries shifted by one (T[u]=LUT[u-1];
        # entry 0 is never read since u >= 1).
        tb = pool.tile([128, 8 * 257], FP32, tag="tb")
        for a in range(8):
            off = 257 * a
            rtop = max(a - 1, 0)
            rbot = a
            if a >= 1:
                nc.sync.dma_start(tb[0:16, off + 1: off + 257], lt[8 * rtop: 8 * rtop + 16, :])
            else:
                for s in range(2):
                    nc.sync.dma_start(tb[8 * s:8 * s + 8, off + 1: off + 257], lt[0:8, :])
            if a <= 6:
                nc.sync.dma_start(tb[64:80, off + 1: off + 257], lt[8 * rbot: 8 * rbot + 16, :])
            else:
                for s in range(2):
                    nc.sync.dma_start(tb[64 + 8 * s: 72 + 8 * s, off + 1: off + 257], lt[56:64, :])
        nc.sync.dma_start(tb[16:32, :], tb[0:16, :])
        nc.sync.dma_start(tb[32:64, :], tb[0:32, :])
        nc.sync.dma_start(tb[80:96, :], tb[64:80, :])
        nc.sync.dma_start(tb[96:128, :], tb[64:96, :])
        return tb

    def apply_stage_idx(ch, a, uidx):
        ustg = pool2.tile([128, W], U16, tag="ustg", name=f"ustg_{ch}_{a}")
        nc.scalar.copy(ustg[:], uidx[:, a * W:(a + 1) * W])
        return ustg

    def apply_gather_half(ch, a, tb, ustg, h, ndve=4):
        """8 chunked gathers (Pool); first 8-ndve converts on Act, rest deferred."""
        gb = pool2.tile([128, 8192], BF16, tag="gb", name=f"gb_{ch}_{a}_{h}")
        gfs = []
        for cc in range(8):
            gf = pool4.tile([128, 1024], FP32, tag="gf", name=f"gf_{ch}_{a}_{h}_{cc}")
            nc.gpsimd.indirect_copy(
                gf[:], tb[:, 257 * a: 257 * a + 257],
                ustg[:, 512 * h + 64 * cc: 512 * h + 64 * cc + 64], True)
            if cc < 8 - ndve:
                nc.scalar.copy(gb[:, 1024 * cc: 1024 * cc + 1024], gf[:])
            else:
                gfs.append((cc, gf))
        return gb, gfs

    def apply_mult_half(ch, a, gb, gfs, h):
        for cc, gf in gfs:
            nc.vector.tensor_copy(gb[:, 1024 * cc: 1024 * cc + 1024], gf[:])
        nc.vector.tensor_tensor(gb[:, :4096], gb[:, :4096],
                                wxb[:, 8192 * h: 8192 * h + 4096], op=OP.mult)
        nc.vector.tensor_tensor(gb[:, 4096:], gb[:, 4096:],
                                wxb[:, 8192 * h + 4096: 8192 * h + 8192], op=OP.mult)
        g3 = gb[:].rearrange("p (x k) -> p x k", k=16)
        pt = []
        for T in range(2):
            t = psred.tile([128, 1536], FP32, tag=f"rT{T}", name=f"rT{T}_{ch}_{a}_{h}")
            pt.append(t)
        for k in range(16):
            q, m = k // 3, k % 3
            T, s = q // 3, q % 3
            nc.tensor.matmul(pt[T][32 * m:32 * m + 8, 512 * s: 512 * s + 512],
                             lab[:, (a * 16 + k) * 8: (a * 16 + k) * 8 + 8],
                             g3[:, :, k], start=True, stop=True)
        return pt

    def apply_out_half(ch, a, pt, h, dve_evac=False):
        f3 = pool2.tile([128, 6 * 512], FP32, tag="f3", name=f"f3_{ch}_{a}_{h}")
        for T in range(2):
            if dve_evac:
                nc.vector.tensor_copy(f3[:, T * 1536: (T + 1) * 1536], pt[T][:])
            else:
                nc.scalar.copy(f3[:, T * 1536: (T + 1) * 1536], pt[T][:])
        for m in range(3):
            nq = 6 if m == 0 else 5
            s0 = f3[32 * m:32 * m + 8, :]
            src2 = dataclasses.replace(s0, ap=[s0.ap[0], [512, nq], [1, 512]])
            dst = dataclasses.replace(
                y_out,
                offset=y_out.offset + (ch * H + 128 * a + m) * W + 512 * h,
                ap=[[16 * W, 8], [3 * W, nq], [1, 512]])
            nc.sync.dma_start(dst, src2)

    def hist_band(ch, a, uidx, hsb, ht):
        hbf, lo = hist_prep(ch, a, uidx)
        hp = pshist.tile([16, 128], FP32, tag="hp")
        for qd in range(4):
            hist_quarter(ch, a, qd, hbf, lo, hp)
        hsb_flush(ch, a, hp, hsb, ht)

    def fused_band(ch_a, a, uidx_a, tb_a, ch_h, h_a, uidx_h, hsb_h, ht_h, preps, ustgs):
        """apply(ch_a, a) interleaved with hist(ch_h, h_a), stage-level order."""
        if a not in ustgs:
            ustgs[a] = apply_stage_idx(ch_a, a, uidx_a)
        ustg = ustgs.pop(a)
        if h_a is not None:
            if h_a not in preps:
                preps[h_a] = hist_prep(ch_h, h_a, uidx_h)
            hbf, lo = preps.pop(h_a)
            hp = pshist.tile([16, 128], FP32, tag="hp")
        gb0, gfs0 = apply_gather_half(ch_a, a, tb_a, ustg, 0)
        if h_a is not None:
            hist_quarter(ch_h, h_a, 0, hbf, lo, hp)
        gb1, gfs1 = apply_gather_half(ch_a, a, tb_a, ustg, 1)
        if h_a is not None:
            hist_quarter(ch_h, h_a, 1, hbf, lo, hp)
            if h_a + 1 < 8:
                preps[h_a + 1] = hist_prep(ch_h, h_a + 1, uidx_h)
        if a + 1 < 8:
            ustgs[a + 1] = apply_stage_idx(ch_a, a + 1, uidx_a)
        if h_a is not None:
            hist_quarter(ch_h, h_a, 2, hbf, lo, hp)
        pt0 = apply_mult_half(ch_a, a, gb0, gfs0, 0)
        if h_a is not None:
            hist_quarter(ch_h, h_a, 3, hbf, lo, hp)
        apply_out_half(ch_a, a, pt0, 0)
        pt1 = apply_mult_half(ch_a, a, gb1, gfs1, 1)
        if h_a is not None:
            hsb_flush(ch_h, h_a, hp, hsb_h, ht_h)
        apply_out_half(ch_a, a, pt1, 1)

    def apply_drain(ch, uidx, tb):
        ustgs = {0: apply_stage_idx(ch, 0, uidx)}
        for a in range(8):
            ustg = ustgs.pop(a)
            gb0, gfs0 = apply_gather_half(ch, a, tb, ustg, 0, ndve=4)
            gb1, gfs1 = apply_gather_half(ch, a, tb, ustg, 1, ndve=4)
            if a + 1 < 8:
                ustgs[a + 1] = apply_stage_idx(ch, a + 1, uidx)
            pt0 = apply_mult_half(ch, a, gb0, gfs0, 0)
            apply_out_half(ch, a, pt0, 0)
            pt1 = apply_mult_half(ch, a, gb1, gfs1, 1)
            apply_out_half(ch, a, pt1, 1)

    # ---- pipeline: hist(ch) -> lut(ch) -> {apply(ch) || hist(ch+1) one band ahead} ----
    cur_u = pool2.tile([128, 8 * W], U8, tag="uidx", name="uidx0")
    cur_s = pool2.tile([16, 8 * 128], FP32, tag="hsb", name="hsb0")
    cur_t = pool2.tile([64, NB], FP32, tag="ht", name="ht0")
    fill_preps = {0: hist_prep(0, 0, cur_u)}
    for a in range(8):
        hbf, lo = fill_preps.pop(a)
        hp = pshist.tile([16, 128], FP32, tag="hp")
        hist_quarter(0, a, 0, hbf, lo, hp)
        hist_quarter(0, a, 1, hbf, lo, hp)
        if a + 1 < 8:
            fill_preps[a + 1] = hist_prep(0, a + 1, cur_u)
        hist_quarter(0, a, 2, hbf, lo, hp)
        hist_quarter(0, a, 3, hbf, lo, hp)
        hsb_flush(0, a, hp, cur_s, cur_t)
    cur_tb = lut_build(0, cur_s, cur_t)
    for ch in range(CH):
        if ch + 1 < CH:
            nxt_u = pool2.tile([128, 8 * W], U8, tag="uidx", name=f"uidx{ch + 1}")
            nxt_s = pool2.tile([16, 8 * 128], FP32, tag="hsb", name=f"hsb{ch + 1}")
            nxt_t = pool2.tile([64, NB], FP32, tag="ht", name=f"ht{ch + 1}")
            preps = {}
            ustgs = {}
            hist_band(ch + 1, 0, nxt_u, nxt_s, nxt_t)
            nxt_tb = None
            for a in range(8):
                fused_band(ch, a, cur_u, cur_tb, ch + 1,
                           a + 1 if a < 7 else None, nxt_u, nxt_s, nxt_t,
                           preps, ustgs)
                if a == 6:
                    nxt_tb = lut_build(ch + 1, nxt_s, nxt_t)
            cur_tb = nxt_tb
            cur_u = nxt_u
        else:
            apply_drain(ch, cur_u, cur_tb)


def _apply_tile_patch():
    """This walrus build rejects >2 sync waits on one instruction; split the
    TileContext exit drain's waits into individual nops."""
    def _patched(self, tick_clock, wait_clock):
        nc = self.nc
        probe = nc.sync.nop()
        wait_clock.add_sem_waits(probe.ins,
                                 tile.ScopedClock({None: tick_clock.global_clock}))
        si = probe.ins.sync_info
        waits = list(si.on_wait) if si and si.on_wait else []
        if len(waits) > 1:
            probe.ins.sync_info = mybir.SyncInfo(on_wait=[waits[0]], on_update=[])
            for w in waits[1:]:
                extra = nc.sync.nop()
                extra.ins.sync_info = mybir.SyncInfo(on_wait=[w], on_update=[])
        nc.sync.drain()
        nc.all_engine_barrier()
        assert self.sems is not None
        popped = nc._tile_sem_poison_stack.pop()
        assert popped is self._sem_poison
        nc.clear_and_free_semaphores(list(self.sems.allocated().values()))
        nc.all_engine_barrier()
    tile.TileContext._drain_and_barrier = _patched


def _split_waits(nc, maxw=1):
    """This container's walrus rejects instructions with more than ~2 sem
    waits; hoist excess waits onto same-engine NoOps inserted just before."""
    import bass_rust
    counter = [0]
    for f in nc.m.functions:
        for blk in f.blocks:
            insts = blk.instructions
            out = []
            for ins in insts:
                si = ins.sync_info
                waits = list(si.on_wait) if si and si.on_wait else []
                if len(waits) > maxw:
                    keep = waits[:maxw]
                    extra = waits[maxw:]
                    for w in extra:
                        counter[0] += 1
                        nop = bass_rust.InstNoOp(
                            name=f"WSPLIT-{counter[0]}", engine=ins.engine,
                            ins=[], outs=[],
                            sync_info=mybir.SyncInfo(on_wait=[w], on_update=[]))
                        out.append(nop)
                    ins.sync_info = mybir.SyncInfo(
                        on_wait=keep, on_update=list(si.on_update or []))
                out.append(ins)
            blk.instructions = out


def build():
    if "nc" in _CACHE:
        return _CACHE["nc"]
    _apply_tile_patch()
    nc = bass.Bass("TRN2", target_bir_lowering=False, debug=False)
    x_in = nc.dram_tensor("x", [CH, H, W], FP32, kind="ExternalInput").ap()
    y_out = nc.dram_tensor("y", [CH, H, W], FP32, kind="ExternalOutput").ap()
    hk = _host_consts()
    K = {k: nc.inline_tensor(v, name=f"const_{k}") for k, v in hk.items()}
    with ExitStack() as ctx:
        tc = ctx.enter_context(tile.TileContext(nc))
        _emit(nc, tc, ctx, x_in, y_out, K)
    _split_waits(nc)
    _CACHE["nc"] = nc
    return nc


def kernel(x: np.ndarray) -> np.ndarray:
    x = np.ascontiguousarray(np.asarray(x, dtype=np.float32))
    assert x.shape == (8, CH, H, W), x.shape
    nc = build()
    in_maps = [{"x": x[i]} for i in range(8)]
    res = run_bass_kernel_spmd(nc, in_maps, list(range(8)))
    out = np.stack([res.results[i]["y"] for i in range(8)], axis=0)
    return out.astype(np.float32)


if __name__ == "__main__":
    x = np.random.rand(8, CH, H, W).astype(np.float32)
    y = kernel(x)
    print("ran:", y.shape, y.dtype)
